# revision 1
# baseline (speedup 1.0000x reference)
"""Trainium2 Bass kernel for nn_DeltaNetLayer (B=4, L=1024, D=256).

Sharding: 8 cores = batch(4) x output-dim-half(2); the delta-rule state W
splits cleanly by output rows (v_bar = W @ phi_k needs only W's own rows).
Each core:
  - projects q,k (full D), its v half, and beta from x_b (fed transposed)
  - phi = LN(elu(.)+1) token-major, PE-transposed per chunk to feature-major
  - chunked recurrence (C=128, 8 chunks): per chunk the unit-lower-triangular
    system (I + diag(b) Gamma.G) Dm = b.(V - df^i K W0^T) is solved exactly
    with the nilpotent product-form inverse J = (I-N)(I+N^2)(I+N^4)(I+N^8)(I+N^16)
    (J is built off the sequential critical path; the carried chain per chunk
    is only KW0 -> rhs -> J@rhs -> W update)
  - final LN over full D needs both halves' stats: pairwise AllReduce of
    per-token (sum, sumsq) split in two rounds so the first hides under the
    remaining chunks' compute; then each core emits a partial output
    projection y_norm[:, half] @ Wo[:, half].T
Host sums the two partials per batch element and adds bo.
"""

import numpy as np

import concourse.bass as bass
import concourse.bacc as bacc
import concourse.mybir as mybir
import concourse.tile as tile
from concourse.bass_utils import run_bass_kernel_spmd

B, L, D = 4, 1024, 256
R = 128           # output-dim rows per core
C = 128           # chunk length (tokens)
NCH = L // C      # 8 chunks
KT = D // 128     # 2 contraction tiles over D
LN_EPS = 1e-5
FP = mybir.dt.float32
FR = mybir.dt.float32r   # single-pass fp32 matmul mode
ALU = mybir.AluOpType
AF = mybir.ActivationFunctionType
AX = mybir.AxisListType.X

REPLICA_GROUPS = [[0, 1], [2, 3], [4, 5], [6, 7]]
USE_FP32R = True

# extra kwargs for run_bass_kernel_spmd (test harness sets trace=True here)
_RUN_KWARGS = {}
_last_results = None


def _host_consts(df, Wq, Wk, beta_w, lnp_g, lnp_b):
    i = np.arange(C)
    pw = i[:, None] - 1 - i[None, :]
    gam = np.where(pw >= 0, df ** np.maximum(pw, 0), 0.0).astype(np.float32)
    consts = {
        "gam": gam,
        "gamT": np.ascontiguousarray(gam.T),
        "ident": np.eye(C, dtype=np.float32),
        # df^i broadcast along partitions (for free-dim scaling of phi_qT)
        "dfB": np.broadcast_to((df ** i).astype(np.float32), (128, C)).copy(),
        # per-partition column vectors: df^i, df^(C-1-i), -df^i
        "dfvec": np.stack(
            [df ** i, df ** (C - 1 - i), -(df ** i)], axis=1
        ).astype(np.float32),
        "wqT": np.ascontiguousarray(
            Wq.T.reshape(KT, 128, D).transpose(1, 0, 2)).astype(np.float32),
        "wkT": np.ascontiguousarray(
            Wk.T.reshape(KT, 128, D).transpose(1, 0, 2)).astype(np.float32),
        "bwT": np.ascontiguousarray(
            beta_w.T.reshape(KT, 128, 1).transpose(1, 0, 2)).astype(np.float32),
        "lnp": np.stack([lnp_g, lnp_b]).astype(np.float32),  # [2, D]
    }
    return consts


def _bcast_ap(src_ap, parts=128):
    """Broadcast a [1, N]-ish AP along the partition dim with stride 0."""
    return bass.AP(
        tensor=src_ap.tensor,
        offset=src_ap.offset,
        ap=[[0, parts], list(src_ap.ap[-1])],
    )


def _build(df, dfC, beta_b, consts, lnp_trivial, ln_trivial):
    nc = bacc.Bacc(
        "TRN2",
        target_bir_lowering=False,
        debug=False,
        num_devices=2 * B,
    )

    MDT = FR if USE_FP32R else FP

    def fp(ap):
        # view an MDT tile as plain f32 for DVE arithmetic
        return ap.bitcast(FP) if USE_FP32R else ap

    # per-core I/O
    xT_d = nc.dram_tensor("xT", [128, KT, L], FP, kind="ExternalInput")
    wvT_d = nc.dram_tensor("wvT", [128, KT, R], FP, kind="ExternalInput")
    woT_d = nc.dram_tensor("woT", [R, D], FP, kind="ExternalInput")
    lngb_d = nc.dram_tensor("lngb", [2, R], FP, kind="ExternalInput")
    out_d = nc.dram_tensor("out_part", [L, D], FP, kind="ExternalOutput")

    # shared constants, baked into the NEFF
    gam_d = nc.inline_tensor(consts["gam"], "c_gam")
    gamT_d = nc.inline_tensor(consts["gamT"], "c_gamT")
    ident_d = nc.inline_tensor(consts["ident"], "c_ident")
    dfB_d = nc.inline_tensor(consts["dfB"], "c_dfB")
    dfvec_d = nc.inline_tensor(consts["dfvec"], "c_dfvec")
    wqT_d = nc.inline_tensor(consts["wqT"], "c_wqT")
    wkT_d = nc.inline_tensor(consts["wkT"], "c_wkT")
    bwT_d = nc.inline_tensor(consts["bwT"], "c_bwT")
    lnp_d = nc.inline_tensor(consts["lnp"], "c_lnp")

    with tile.TileContext(nc) as tc:
        with (
            tc.tile_pool(name="const", bufs=1) as pc,
            tc.tile_pool(name="pers", bufs=1) as pp,
            tc.tile_pool(name="scr", bufs=3) as ps,
            tc.tile_pool(name="scr2", bufs=2) as ps2,
            tc.tile_pool(name="psproj", bufs=2, space="PSUM") as ppj,
            tc.tile_pool(name="psprep", bufs=3, space="PSUM") as ppr,
            tc.tile_pool(name="pschain", bufs=3, space="PSUM") as pch,
            tc.tile_pool(name="dram", bufs=1, space="DRAM") as pd,
        ):
            # ---------------- constant / weight loads ----------------
            def ctile(nm, shape, src):
                t = pc.tile(shape, FP, name=nm)
                nc.gpsimd.dma_start(out=t[:], in_=src)
                return t

            def ctile_r(nm, shape, src):
                # DMA to f32 staging, then round to MDT (fp32r needs
                # producer-side rounding for matmul operands)
                t = pc.tile(shape, FP, name=nm + "_f")
                nc.gpsimd.dma_start(out=t[:], in_=src)
                if not USE_FP32R:
                    return t
                tr = pc.tile(shape, MDT, name=nm)
                nc.vector.tensor_copy(tr[:], t[:])
                return tr

            gam = ctile("gam", [C, C], gam_d[:, :])
            gamT = ctile("gamT", [C, C], gamT_d[:, :])
            ident = ctile_r("ident", [C, C], ident_d[:, :])
            dfB = ctile("dfB", [128, C], dfB_d[:, :])
            dfvec = ctile("dfvec", [128, 3], dfvec_d[:, :])
            wq = ctile_r("wq", [128, KT, D], wqT_d[:, :, :])
            wk = ctile_r("wk", [128, KT, D], wkT_d[:, :, :])
            bw = ctile("bw", [128, KT, 1], bwT_d[:, :, :])
            wv = ctile_r("wv", [128, KT, R], wvT_d[:, :, :])
            wo = ctile_r("wo", [R, D], woT_d[:, :])
            if not lnp_trivial:
                lnpg = ctile("lnpg", [128, D], _bcast_ap(lnp_d[0, :]))
                lnpb = ctile("lnpb", [128, D], _bcast_ap(lnp_d[1, :]))
            if not ln_trivial:
                lngB = ctile("lngB", [128, R], _bcast_ap(lngb_d[0, :]))
                lnbB = ctile("lnbB", [128, R], _bcast_ap(lngb_d[1, :]))
            eps_t = pc.tile([128, 1], FP)
            nc.vector.memset(eps_t[:], LN_EPS)
            nbb_t = pc.tile([128, 1], FP)
            nc.vector.memset(nbb_t[:], -float(beta_b))
            xt_f = pc.tile([128, KT, L], FP)
            xt = pc.tile([128, KT, L], MDT, name="xt") if USE_FP32R else xt_f
            for c in range(NCH):
                nc.gpsimd.dma_start(
                    out=xt_f[:, :, c * C:(c + 1) * C],
                    in_=xT_d[:, :, c * C:(c + 1) * C],
                )
                if USE_FP32R:
                    nc.vector.tensor_copy(xt[:, :, c * C:(c + 1) * C],
                                          xt_f[:, :, c * C:(c + 1) * C])

            # ---------------- persistent per-chunk storage ----------------
            phiq = pp.tile([128, NCH, D], MDT)     # token-major phi_q
            phik = pp.tile([128, NCH, D], MDT)
            # feature-major [k^T | q^T] adjacent so G and A share one matmul
            phikq = pp.tile([128, KT, NCH, 2 * C], MDT)
            phiqTs = pp.tile([128, KT, NCH, C], MDT)  # df^i-scaled q^T
            kps = pp.tile([128, NCH, D], MDT)      # df^(C-1-i)-scaled k
            bV = pp.tile([128, NCH, R], MDT)
            bcol = pp.tile([128, NCH], FP)        # beta
            nbdf = pp.tile([128, NCH], FP)        # -beta*df^i
            JTs = pp.tile([128, NCH, C], MDT)
            ATs = pp.tile([128, NCH, C], MDT)
            ys = pp.tile([128, NCH, R], FP)
            s1 = pp.tile([128, NCH], FP)
            s2 = pp.tile([128, NCH], FP)
            mu = pp.tile([128, NCH], FP)
            rstd = pp.tile([128, NCH], FP)
            w_state = [pp.tile([128, KT, R], MDT, name=f"w{i}")
                       for i in range(2)]

            def csl(c):
                return slice(c * C, (c + 1) * C)

            def mm(out, lhsT, rhs, **kw):
                nc.tensor.matmul(out, lhsT=lhsT, rhs=rhs, **kw)

            def tp(out, in_):
                nc.tensor.transpose(out, in_, ident[:])

            # ---------------- projections (pairs of chunks; ACT ops
            # batched by function to avoid table thrash) -------------------
            def proj_pair(c0):
                chunks = [c for c in (c0, c0 + 1) if c < NCH]
                pre, mvs = {}, {}
                for c in chunks:
                    sl = csl(c)
                    for nm, w_sb in (("q", wq), ("k", wk)):
                        pj = ppj.tile([128, D], FP, tag="proj")
                        mm(pj[:], xt[:, 0, sl], w_sb[:, 0, :],
                           start=True, stop=False)
                        mm(pj[:], xt[:, 1, sl], w_sb[:, 1, :],
                           start=False, stop=True)
                        # elu+1 = relu(x) + exp(min(x,0))
                        e_t = ps.tile([128, D], FP, tag=f"elu_e{nm}{c % 2}")
                        nc.vector.tensor_scalar_min(e_t[:], pj[:], 0.0)
                        nc.scalar.activation(e_t[:], e_t[:], AF.Exp)
                        r_t = ps.tile([128, D], FP, tag=f"elu_r{nm}{c % 2}")
                        nc.vector.tensor_scalar_max(r_t[:], pj[:], 0.0)
                        nc.vector.tensor_add(r_t[:], r_t[:], e_t[:])
                        st6 = ps.tile([128, 6], FP, tag=f"st6{nm}{c % 2}")
                        mv = ps.tile([128, 2], FP, tag=f"mv{nm}{c % 2}")
                        nc.vector.bn_stats(out=st6[:], in_=r_t[:])
                        nc.vector.bn_aggr(out=mv[:], in_=st6[:])
                        pre[(c, nm)] = r_t
                        mvs[(c, nm)] = mv
                    # v half (scaled by beta below)
                    pv = ppj.tile([128, R], FP, tag="proj")
                    mm(pv[:], xt[:, 0, sl], wv[:, 0, :], start=True, stop=False)
                    mm(pv[:], xt[:, 1, sl], wv[:, 1, :], start=False, stop=True)
                    # beta = 1/(1+exp(-(x@bw^T+bb)))  (reuses the Exp table)
                    pb = ppj.tile([128, 1], FP, tag="proj")
                    nc.tensor.matmul(pb[:], lhsT=xt_f[:, 0, sl],
                                     rhs=bw[:, 0, :], start=True, stop=False)
                    nc.tensor.matmul(pb[:], lhsT=xt_f[:, 1, sl],
                                     rhs=bw[:, 1, :], start=False, stop=True)
                    bexp = ps.tile([128, 1], FP, tag="bexp")
                    nc.scalar.activation(bexp[:], pb[:], AF.Exp,
                                         bias=nbb_t[:], scale=-1.0)
                    nc.vector.tensor_scalar_add(bexp[:], bexp[:], 1.0)
                    nc.vector.reciprocal(bcol[:, c:c + 1], bexp[:])
                    nc.vector.tensor_mul(nbdf[:, c:c + 1], bcol[:, c:c + 1],
                                         dfvec[:, 2:3])
                    nc.vector.tensor_scalar_mul(bV[:, c, :], pv[:],
                                                bcol[:, c:c + 1])
                # batched Sqrt for the pair (one ACT table context)
                for c in chunks:
                    for nm, dst in (("q", phiq), ("k", phik)):
                        mv = mvs[(c, nm)]
                        sd = ps.tile([128, 1], FP, tag=f"sd{nm}{c % 2}")
                        rsd = ps.tile([128, 1], FP, tag=f"rsd{nm}{c % 2}")
                        nc.scalar.activation(sd[:], mv[:, 1:2], AF.Sqrt,
                                             bias=eps_t[:])
                        nc.vector.reciprocal(rsd[:], sd[:])
                        nc.vector.tensor_scalar(
                            out=dst[:, c, :], in0=pre[(c, nm)][:],
                            scalar1=mv[:, 0:1], scalar2=rsd[:],
                            op0=ALU.subtract, op1=ALU.mult)
                        if not lnp_trivial:
                            nc.vector.tensor_mul(dst[:, c, :],
                                                 fp(dst[:, c, :]), lnpg[:])
                            nc.vector.tensor_add(dst[:, c, :],
                                                 fp(dst[:, c, :]), lnpb[:])
                for c in chunks:
                    nc.vector.tensor_scalar_mul(kps[:, c, :],
                                                fp(phik[:, c, :]),
                                                dfvec[:, 1:2])

            # ---------------- per-chunk prep: transposes, G|A, N ----------
            def prep_a(c):
                for src, off in ((phik, 0), (phiq, C)):
                    for kt in range(KT):
                        pt = ppr.tile([128, 128], MDT, tag="prep")
                        tp(pt[:], src[:, c, kt * 128:(kt + 1) * 128])
                        nc.vector.tensor_copy(
                            phikq[:, kt, c, off:off + C], fp(pt[:]))
                for kt in range(KT):
                    nc.vector.tensor_mul(phiqTs[:, kt, c, :],
                                         fp(phikq[:, kt, c, C:2 * C]), dfB[:])
                # [G | A_raw] = K^T.T @ [K^T | Q^T] in one accumulation group
                pg = ppr.tile([128, 2 * C], FP, tag="prep")
                mm(pg[:], phikq[:, 0, c, 0:C], phikq[:, 0, c, :],
                   start=True, stop=False)
                mm(pg[:], phikq[:, 1, c, 0:C], phikq[:, 1, c, :],
                   start=False, stop=True)
                # AT = (K Q^T) o Gamma^T ; N = b_i Gamma o G
                nc.vector.tensor_mul(ATs[:, c, :], fp(pg[:, C:2 * C]), gamT[:])
                n_t = ps2.tile([128, C], MDT, tag="n")
                nc.vector.scalar_tensor_tensor(
                    out=n_t[:], in0=fp(pg[:, 0:C]), scalar=bcol[:, c:c + 1],
                    in1=gam[:], op0=ALU.mult, op1=ALU.mult)
                ptr = ppr.tile([128, C], MDT, tag="prep")
                tp(ptr[:], n_t[:])
                nt_t = ps2.tile([128, C], MDT, tag="nt")
                nc.vector.tensor_copy(nt_t[:], fp(ptr[:]))
                return n_t, nt_t

            def prep_b(c, n_t, nt_t):
                # J = (I-N)(I+N^2)(I+N^4)(I+N^8)(I+N^16) via repeated squaring
                jt_cur = ps2.tile([128, C], MDT, tag="jt")
                nc.vector.tensor_sub(jt_cur[:], fp(ident[:]), fp(nt_t[:]))
                s_cur, st_cur = n_t, nt_t
                for lvl in range(4):
                    ps_a = ppr.tile([128, C], FP, tag="prep")
                    mm(ps_a[:], st_cur[:], s_cur[:], start=True, stop=True)
                    s_new = ps2.tile([128, C], MDT, tag=f"s{lvl}")
                    nc.vector.tensor_copy(s_new[:], ps_a[:])
                    if lvl < 3:
                        ps_b = ppr.tile([128, C], FP, tag="prep")
                        mm(ps_b[:], s_cur[:], st_cur[:], start=True, stop=True)
                        st_new = ps2.tile([128, C], MDT, tag=f"st{lvl}")
                        nc.vector.tensor_copy(st_new[:], ps_b[:])
                    else:
                        st_new = None
                    pj_f = ppr.tile([128, C], FP, tag="prep")
                    mm(pj_f[:], s_new[:], jt_cur[:], start=True, stop=True)
                    if lvl < 3:
                        jt_new = ps2.tile([128, C], MDT, tag=f"jt{lvl}")
                        nc.vector.tensor_add(jt_new[:], fp(jt_cur[:]), pj_f[:])
                        jt_cur = jt_new
                        s_cur, st_cur = s_new, st_new
                    else:
                        nc.vector.tensor_add(JTs[:, c, :], fp(jt_cur[:]), pj_f[:])

            # ---------------- sequential chain ----------------
            def chain_a(c):
                if c == 0:
                    return None
                w_prev = w_state[(c + 1) % 2]
                pkw = pch.tile([128, R], FP, tag="chain")
                mm(pkw[:], phikq[:, 0, c, 0:C], w_prev[:, 0, :],
                   start=True, stop=False)
                mm(pkw[:], phikq[:, 1, c, 0:C], w_prev[:, 1, :],
                   start=False, stop=True)
                x_t = ps.tile([128, R], MDT, tag="xrhs")
                nc.vector.scalar_tensor_tensor(
                    out=x_t[:], in0=pkw[:], scalar=nbdf[:, c:c + 1],
                    in1=fp(bV[:, c, :]), op0=ALU.mult, op1=ALU.add)
                return x_t

            def chain_b(c, x_t):
                pdm = pch.tile([128, R], FP, tag="chain")
                mm(pdm[:], JTs[:, c, :], (x_t[:] if c > 0 else bV[:, c, :]),
                   start=True, stop=True)
                dm = ps.tile([128, R], MDT, tag="dm")
                nc.vector.tensor_copy(dm[:], pdm[:])
                return dm

            def chain_c(c, dm):
                w_prev = w_state[(c + 1) % 2]
                w_next = w_state[c % 2]
                po = pch.tile([128, R], FP, tag="chain")
                if c > 0:
                    mm(po[:], phiqTs[:, 0, c, :], w_prev[:, 0, :],
                       start=True, stop=False)
                    mm(po[:], phiqTs[:, 1, c, :], w_prev[:, 1, :],
                       start=False, stop=False)
                    mm(po[:], ATs[:, c, :], dm[:], start=False, stop=True)
                else:
                    mm(po[:], ATs[:, c, :], dm[:], start=True, stop=True)
                nc.vector.tensor_copy(ys[:, c, :], po[:])
                for kt in range(KT):
                    pw = pch.tile([128, R], FP, tag="chain")
                    mm(pw[:], kps[:, c, kt * 128:(kt + 1) * 128], dm[:],
                       start=True, stop=True)
                    if c > 0:
                        nc.vector.scalar_tensor_tensor(
                            out=w_next[:, kt, :], in0=fp(w_prev[:, kt, :]),
                            scalar=float(dfC), in1=pw[:],
                            op0=ALU.mult, op1=ALU.add)
                    else:
                        nc.vector.tensor_copy(w_next[:, kt, :], pw[:])
                # final-LN partial stats
                nc.vector.reduce_sum(out=s1[:, c:c + 1], in_=ys[:, c, :],
                                     axis=AX)
                sq_t = ps.tile([128, R], FP, tag="sq")
                nc.scalar.activation(sq_t[:], ys[:, c, :], AF.Square,
                                     accum_out=s2[:, c:c + 1])

            # ---------------- split collective + LN stats -----------------
            def collective_half(h):
                lo, hi = h * 4, h * 4 + 4
                cin = pd.tile([2, 4 * C], FP, name=f"ccin{h}")
                cout = pd.tile([2, 4 * C], FP, name=f"ccout{h}")
                nc.gpsimd.dma_start(
                    out=cin[0, :].rearrange("(c p) -> p c", p=128),
                    in_=s1[:, lo:hi])
                nc.gpsimd.dma_start(
                    out=cin[1, :].rearrange("(c p) -> p c", p=128),
                    in_=s2[:, lo:hi])
                nc.gpsimd.collective_compute(
                    "AllReduce", ALU.add, replica_groups=REPLICA_GROUPS,
                    ins=[cin.opt()], outs=[cout.opt()])
                s1b = ps2.tile([128, 4], FP, tag=f"s1b{h}")
                s2b = ps2.tile([128, 4], FP, tag=f"s2b{h}")
                nc.gpsimd.dma_start(
                    out=s1b[:], in_=cout[0, :].rearrange("(c p) -> p c", p=128))
                nc.gpsimd.dma_start(
                    out=s2b[:], in_=cout[1, :].rearrange("(c p) -> p c", p=128))
                m_t = mu[:, lo:hi]
                v_t = ps2.tile([128, 4], FP, tag=f"var{h}")
                m2_t = ps2.tile([128, 4], FP, tag=f"m2{h}")
                nc.vector.tensor_scalar_mul(m_t, s1b[:], 1.0 / D)
                nc.vector.tensor_scalar_mul(v_t[:], s2b[:], 1.0 / D)
                nc.vector.tensor_mul(m2_t[:], m_t, m_t)
                nc.vector.tensor_sub(v_t[:], v_t[:], m2_t[:])
                nc.scalar.activation(v_t[:], v_t[:], AF.Sqrt, bias=eps_t[:])
                nc.vector.reciprocal(rstd[:, lo:hi], v_t[:])

            # ---------------- final: normalize + partial out --------------
            out_ap = out_d[:, :].rearrange("(c p) d -> p c d", p=128)

            def final_chunk(c):
                yn = ps.tile([128, R], MDT, tag="yn")
                nc.vector.tensor_scalar(
                    out=yn[:], in0=ys[:, c, :], scalar1=mu[:, c:c + 1],
                    scalar2=rstd[:, c:c + 1], op0=ALU.subtract, op1=ALU.mult)
                if not ln_trivial:
                    nc.vector.tensor_mul(yn[:], fp(yn[:]), lngB[:])
                    nc.vector.tensor_add(yn[:], fp(yn[:]), lnbB[:])
                ptp = ppr.tile([128, 128], MDT, tag="prep")
                tp(ptp[:], yn[:])
                ynT = ps.tile([128, R], MDT, tag="ynT")
                nc.vector.tensor_copy(ynT[:], fp(ptp[:]))
                pf = pch.tile([128, D], FP, tag="chain")
                mm(pf[:], ynT[:], wo[:], start=True, stop=True)
                ostg = ps.tile([128, D], FP, tag="ostg")
                nc.vector.tensor_copy(ostg[:], pf[:])
                nc.gpsimd.dma_start(out=out_ap[:, c, :], in_=ostg[:])

            # ---------------- emission ----------------
            proj_pair(0)
            na = prep_a(0)
            prep_b(0, *na)
            proj_pair(2)
            for c in range(NCH):
                if c + 1 < NCH:
                    na = prep_a(c + 1)
                x_t = chain_a(c)
                if c + 4 < NCH and c % 2 == 0:
                    proj_pair(c + 4)
                dm = chain_b(c, x_t)
                if c + 1 < NCH:
                    prep_b(c + 1, *na)
                chain_c(c, dm)
                if c == 3:
                    collective_half(0)
                if c == 5:
                    for fc in range(4):
                        final_chunk(fc)
            collective_half(1)
            for fc in range(4, NCH):
                final_chunk(fc)

    nc.compile()
    return nc


def kernel(**inputs):
    x = np.ascontiguousarray(np.asarray(inputs["x"], np.float32))
    Wq = np.asarray(inputs["Wq"], np.float32)
    Wk = np.asarray(inputs["Wk"], np.float32)
    Wv = np.asarray(inputs["Wv"], np.float32)
    beta_w = np.asarray(inputs["beta_w"], np.float32)
    beta_b = np.asarray(inputs["beta_b"], np.float32)
    decay = np.asarray(inputs["decay"], np.float32)
    Wo = np.asarray(inputs["Wo"], np.float32)
    bo = np.asarray(inputs["bo"], np.float32)
    ln_g = np.asarray(inputs["ln_g"], np.float32)
    ln_b = np.asarray(inputs["ln_b"], np.float32)
    lnp_g = np.asarray(inputs["lnp_g"], np.float32)
    lnp_b = np.asarray(inputs["lnp_b"], np.float32)

    df = float(1.0 / (1.0 + np.exp(-float(decay[0]))))
    dfC = df ** C
    lnp_trivial = bool(np.all(lnp_g == 1.0) and np.all(lnp_b == 0.0))
    ln_trivial = bool(np.all(ln_g == 1.0) and np.all(ln_b == 0.0))
    consts = _host_consts(df, Wq, Wk, beta_w, lnp_g, lnp_b)
    nc = _build(df, dfC, float(beta_b[0]), consts, lnp_trivial, ln_trivial)

    in_maps = []
    for b in range(B):
        xT = np.ascontiguousarray(x[b].T.reshape(KT, 128, L).transpose(1, 0, 2))
        for h in range(2):
            rs = slice(h * R, (h + 1) * R)
            in_maps.append({
                "xT": xT,
                "wvT": np.ascontiguousarray(
                    Wv[rs, :].T.reshape(KT, 128, R).transpose(1, 0, 2)),
                "woT": np.ascontiguousarray(Wo[:, rs].T),
                "lngb": np.stack([ln_g[rs], ln_b[rs]]).astype(np.float32),
            })

    res = run_bass_kernel_spmd(nc, in_maps, core_ids=list(range(2 * B)),
                               **_RUN_KWARGS)
    globals()["_last_results"] = res
    out = np.zeros((B, L, D), np.float32)
    for b in range(B):
        out[b] = res.results[2 * b]["out_part"] + res.results[2 * b + 1]["out_part"]
        out[b] += bo[None, :]
    return out

